# revision 21
# baseline (speedup 1.0000x reference)
"""Multi-head causal self-attention on 8 Trainium2 NeuronCores.

Sharding: tensor-parallel over heads (4 heads/core) x data-parallel over
batch (B=2): core c -> batch c//4, head-group c%4. Each core computes its
4 heads' attention plus a partial output projection; the host sums the 4
partials per batch element.

Layout strategy (per core):
  - x is fed pre-transposed (xT: [D, T]) so QKV projections produce
    qT/kT ([head_dim, T], head-dim on partitions) and v ([T, head_dim])
    directly, with no on-device transposes anywhere.
  - Startup: the critical-path DMA (wk/wq/xc-block0 in k-chunk order,
    rotated across three issue queues) goes out first so the QKV matmuls
    start as soon as chunk 0 lands and stream behind the DMA; wv / the
    other x blocks / wo / normalization constants follow in need-order.
  - Scores are computed transposed (k on partitions, q on free dim):
    psum[k, q] = kT_tile.T @ qT_block. Two heads run concurrently via
    row-tiled tile_position (dk=64 each) into one 2-bank psum tile, so
    one Exp activation covers the pair (halves ACT call overhead).
  - Softmax skips max-subtraction (scores are bounded well inside fp32
    exp range); exp runs on ScalarE with scale=1/sqrt(dk) folded in.
    Causal masking multiplies only diagonal tiles by a 0/1 mask, one
    head on VectorE and one on GpSimd.
  - P@V uses an M=65 stationary [v_head | ones] so the softmax
    denominators accumulate in psum row 64 of the same matmul.
  - Normalization: denominator rows are bounced via tiny DMAs into
    collector tiles, inverted with the fast Newton-Raphson reciprocal,
    broadcast across partitions by a one-hot-selector matmul, and
    multiplied in.
  - The exp chain paces the attention phase, so the PE stream is kept
    stall-free to hold the HAM clock at 2.4GHz: the m-loop is software
    pipelined (scores of step m+1 issue before P@V of step m), the two
    head pairs' blocks alternate, and QKV / output-projection matmul
    groups are interleaved into the attention phase as PE filler.
  - Matmul operands are bf16 (1 cycle/column on the PE; fp32r costs 2).
    Accumulation is fp32 in PSUM; denominators/reciprocals stay fp32.
  - y partials leave as bf16 (halves output DMA); the host sums in fp32.
"""

import sys

for _p in ("/opt/trn_rl_repo",):
    if _p not in sys.path:
        sys.path.append(_p)

import numpy as np

P = 128
T = 2048
D = 1024
OD = 256  # output dims per core = 4 heads x 64
DK = 64
NQ = 512  # q-block (psum free size)
N_CORES = 8

_CACHE = {}


def _build_nc(t=T, d=D, od=OD):
    import concourse.bass as bass
    import concourse.tile as tile
    from concourse import bacc, mybir

    f32 = mybir.dt.float32
    f32r = mybir.dt.float32r
    bf16 = mybir.dt.bfloat16

    kt = d // P        # k-tiles over d_model
    tt = t // P        # token tiles
    nb = t // NQ       # q blocks
    npair = od // P    # head pairs (2 heads per 128 partitions)
    dpb = NQ // P      # diagonal k-tiles per q block
    nh = od // DK      # heads per core

    nslotsA = 2 * npair * (nb - 1)  # (pair, j, head) slots with j < nb-1
    nslotsB = 2 * npair             # slots with j == nb-1
    nrows = max(nslotsA, 1)
    nrowsB = 32 * (nslotsB - 1) + 1  # batch-B rows live at partitions 32*i

    nc = bacc.Bacc("TRN2", target_bir_lowering=False, debug=False)

    xT = nc.dram_tensor("xT", [d, t], bf16, kind="ExternalInput")
    wqT = nc.dram_tensor("wqT", [d, od], bf16, kind="ExternalInput")
    wkT = nc.dram_tensor("wkT", [d, od], bf16, kind="ExternalInput")
    wvT = nc.dram_tensor("wvT", [d, od], bf16, kind="ExternalInput")
    woT = nc.dram_tensor("woT", [od, d], bf16, kind="ExternalInput")
    masks = nc.dram_tensor("masks", [P, P], bf16, kind="ExternalInput")
    emat = nc.dram_tensor("emat", [nrows, nrows * DK], bf16, kind="ExternalInput")
    ematB = nc.dram_tensor("ematB", [nrowsB, nslotsB * DK], f32r, kind="ExternalInput")
    y = nc.dram_tensor("y", [t, d], bf16, kind="ExternalOutput")

    Exp = mybir.ActivationFunctionType.Exp
    scale = 1.0 / float(np.sqrt(DK))

    with tile.TileContext(nc) as tc:
        with (
            tc.tile_pool(name="const", bufs=1) as cpool,
            tc.tile_pool(name="qk", bufs=2 * npair * nb) as qkpool,
            tc.tile_pool(name="vp", bufs=tt) as vpool,
            tc.tile_pool(name="ht", bufs=npair * nb) as hpool,
            tc.tile_pool(name="hu", bufs=2 * npair * nb) as hupool,
            tc.tile_pool(name="work", bufs=8) as wpool,
            tc.tile_pool(name="psS", bufs=2, space="PSUM") as psS,
            tc.tile_pool(name="psH", bufs=2, space="PSUM") as psH,
            tc.tile_pool(name="psF", bufs=2, space="PSUM") as psF,
        ):
            # ---- constant tiles (DMAs for the late-needed ones issue last) ----
            wo_sb = cpool.tile([P, npair * d], bf16, tag="wo")
            mask_sb = cpool.tile([P, P], bf16, tag="mask")
            emat_sb = cpool.tile([nrows, nrows * DK], bf16, tag="emat")
            ematB_sb = cpool.tile([nrowsB, nslotsB * DK], f32r, tag="ematB")

            # x and the QKV weights live in a scoped pool released after the
            # projections, freeing space for the attention phase.
            xpool = tc.alloc_tile_pool(name="xp", bufs=1)
            xc = [xpool.tile([P, kt * NQ], bf16, tag=f"xc{c}", name=f"xc_{c}") for c in range(nb)]
            wq_sb = xpool.tile([P, kt * od], bf16, tag="wq")
            wk_sb = xpool.tile([P, kt * od], bf16, tag="wk")
            wv_sb = xpool.tile([P, kt * od], bf16, tag="wv")

            _eng = [nc.sync, nc.gpsimd, nc.scalar]
            _ei = [0]

            def _issue(dst, src):
                _eng[_ei[0] % 3].dma_start(dst, src)
                _ei[0] += 1

            xTv = xT.rearrange("(k p) q -> p k q", p=P)
            wqv = wqT.rearrange("(k p) o -> p k o", p=P)
            wkv = wkT.rearrange("(k p) o -> p k o", p=P)
            wvv = wvT.rearrange("(k p) o -> p k o", p=P)
            xdst = [xc[c][:].rearrange("p (k q) -> p k q", q=NQ) for c in range(nb)]
            wqd = wq_sb[:].rearrange("p (k o) -> p k o", o=od)
            wkd = wk_sb[:].rearrange("p (k o) -> p k o", o=od)
            wvd = wv_sb[:].rearrange("p (k o) -> p k o", o=od)

            # mask first (needed at the very first diagonal step; tiny)
            nc.sync.dma_start(mask_sb[:], masks[:])
            # critical path: wk/wq/xc0 k-chunks, earliest k first, spread so
            # every queue carries a mix (the first matmul waits only on the
            # first chunks of each)
            for i, kp in enumerate(range(0, kt, 2)):
                _eng[(i + 0) % 3].dma_start(wkd[:, kp:kp + 2], wkv[:, kp:kp + 2])
                _eng[(i + 1) % 3].dma_start(wqd[:, kp:kp + 2], wqv[:, kp:kp + 2])
                _eng[(i + 2) % 3].dma_start(xdst[0][:, kp:kp + 2], xTv[:, kp:kp + 2, 0:NQ])
            # wv next (v-projection warm-up), then the remaining x blocks,
            # then output-side constants
            for kp in range(0, kt, 4):
                _issue(wvd[:, kp:kp + 4], wvv[:, kp:kp + 4])
            for c in range(1, nb):
                for h0 in range(0, kt, 4):
                    _issue(xdst[c][:, h0:h0 + 4], xTv[:, h0:h0 + 4, c * NQ:(c + 1) * NQ])
            for pp in range(npair):
                _issue(wo_sb[:, pp * d:(pp + 1) * d], woT[pp * P:(pp + 1) * P, :])
            _issue(emat_sb[:], emat[:])
            _issue(ematB_sb[:], ematB[:])

            # ---- persistent tiles ----
            qT = [[qkpool.tile([P, NQ], bf16, tag="qT", name=f"qT_{pp}_{n}") for n in range(nb)] for pp in range(npair)]
            kT = [[qkpool.tile([P, NQ], bf16, tag="kT", name=f"kT_{pp}_{n}") for n in range(nb)] for pp in range(npair)]
            v_sb = [vpool.tile([P, nh * (DK + 1)], bf16, tag="v", name=f"v_{tk}") for tk in range(tt)]
            hT = [[hpool.tile([P, NQ], bf16, tag="hT", name=f"hT_{pp}_{n}") for n in range(nb)] for pp in range(npair)]
            hu = {}

            sumsA = wpool.tile([max(nslotsA, 1), NQ], f32, tag="sumsA", bufs=1)
            sumsB = wpool.tile([nrowsB, NQ], f32, tag="sumsB", bufs=1)
            nc.vector.memset(sumsB[:], 1.0)
            batchA = []  # (pp, j, h) in collector-row order
            batchB = []

            # ---- emit helpers ----
            def emit_kq(pp, n, which=(0, 1)):
                for idx, (dst, w_sb) in enumerate(((kT, wk_sb), (qT, wq_sb))):
                    if idx not in which:
                        continue
                    ps = psF.tile([P, NQ], f32, tag="acc", name=f"kqps_{pp}_{n}_{idx}")
                    for k in range(kt):
                        nc.tensor.matmul(
                            ps[:],
                            w_sb[:, k * od + pp * P: k * od + (pp + 1) * P],
                            xc[n][:, k * NQ:(k + 1) * NQ],
                            start=(k == 0),
                            stop=(k == kt - 1),
                        )
                    nc.vector.tensor_copy(dst[pp][n][:], ps[:])

            def emit_v(tk):
                # each head's 64 v-columns are followed by a ones column so
                # the P@V matmul also accumulates the softmax denominator
                vv = v_sb[tk][:].rearrange("p (h c) -> p h c", c=DK + 1)
                nc.vector.memset(v_sb[tk][:], 1.0)
                ps = psF.tile([P, od], f32, tag="acc", name=f"vps_{tk}")
                for k in range(kt):
                    nc.tensor.matmul(
                        ps[:],
                        xc[tk // dpb][:, k * NQ + (tk % dpb) * P: k * NQ + (tk % dpb + 1) * P],
                        wv_sb[:, k * od:(k + 1) * od],
                        start=(k == 0),
                        stop=(k == kt - 1),
                    )
                nc.vector.tensor_copy(
                    vv[:, :, 0:DK],
                    ps[:].rearrange("p (h c) -> p h c", c=DK),
                )

            invA_holder = {}

            def emit_normA_recip(nslots):
                invf = wpool.tile([nslots, NQ], f32, tag="invAf", bufs=1, name="inv_Af")
                nc.vector.reciprocal_approx_fast(invf[:], sumsA[0:nslots, :])
                inv = wpool.tile([nslots, NQ], bf16, tag="invA", bufs=1, name="inv_A")
                nc.vector.tensor_copy(inv[:], invf[:])
                invA_holder["inv"] = inv

            def emit_normA_slot(i, nslots):
                pp, j, h = batchA[i]
                inv = invA_holder["inv"]
                psb = psF.tile([DK, NQ], f32, tag="acc", name=f"psb_A_{i}")
                nc.tensor.matmul(psb[:], emat_sb[0:nslots, i * DK:(i + 1) * DK], inv[:], start=True, stop=True)
                nc.vector.tensor_mul(hT[pp][j][h * DK:(h + 1) * DK, :], hu[(pp, j, h)][:], psb[:])

            def emit_normB(part, lo_i, ps_pool=None):
                rows = 32 * (len(part) - 1) + 1
                invB = wpool.tile([rows, NQ], f32r, tag="invB", bufs=2, name=f"invB_{lo_i}")
                with nc.allow_low_precision(reason="f32r shares f32 bits"):
                    nc.vector.reciprocal(invB[:], sumsB[32 * lo_i:32 * lo_i + rows, :])
                for i, (pp, j, h) in enumerate(part):
                    psb = (ps_pool or psF).tile([DK, NQ], f32, tag="acc", name=f"psbB_{lo_i}_{i}")
                    nc.tensor.matmul(psb[:], ematB_sb[0:rows, i * DK:(i + 1) * DK], invB[:], start=True, stop=True)
                    nc.vector.tensor_mul(hT[pp][j][h * DK:(h + 1) * DK, :], hu[(pp, j, h)][:], psb[:])

            obw = min(NQ, d)

            def emit_oproj(tk, use_act=False):
                for ob in range(d // obw):
                    psy = psF.tile([P, obw], f32, tag="acc", name=f"yps_{tk}_{ob}")
                    for pp in range(npair):
                        nc.tensor.matmul(
                            psy[:],
                            hT[pp][tk // dpb][:, (tk % dpb) * P:(tk % dpb + 1) * P],
                            wo_sb[:, pp * d + ob * obw: pp * d + (ob + 1) * obw],
                            start=(pp == 0),
                            stop=(pp == npair - 1),
                        )
                    ysb = wpool.tile([P, obw], bf16, tag="ysb", bufs=4)
                    if use_act:
                        nc.scalar.copy(ysb[:], psy[:])
                    else:
                        nc.vector.tensor_copy(ysb[:], psy[:])
                    nc.sync.dma_start(y[tk * P:(tk + 1) * P, ob * obw:(ob + 1) * obw], ysb[:])

            # ---- warm-up: just enough for (pair 0, block 0) ----
            emit_kq(0, 0)
            for tk in range(min(dpb, tt)):
                emit_v(tk)

            # ---- attention m-step: both heads' scores land in one 2-bank
            # psum tile so a single Exp covers the pair; only the 128-wide
            # diagonal boundary needs the causal mask ----
            def emit_step(pp, j, m):
                dlt = m - dpb * j
                lo = max(dlt, 0) * P  # first live q column of this k-tile
                pss = psS.tile([P, 2 * NQ], f32, tag="acc", name=f"pss_{pp}_{j}_{m}")
                ps3 = pss[:].rearrange("p (h q) -> p h q", q=NQ)
                for h in range(2):
                    nc.tensor.matmul(
                        ps3[:, h, lo:],
                        kT[pp][m // dpb][h * DK:(h + 1) * DK, (m % dpb) * P:(m % dpb + 1) * P],
                        qT[pp][j][h * DK:(h + 1) * DK, lo:],
                        start=True,
                        stop=True,
                        tile_position=(h * DK, 0),
                    )
                e = wpool.tile([P, 2 * NQ], bf16, tag="exp")
                e3 = e[:].rearrange("p (h q) -> p h q", q=NQ)
                nc.scalar.activation(e3[:, :, lo:], ps3[:, :, lo:], Exp, bias=0.0, scale=scale)
                if dlt >= 0:
                    nc.vector.tensor_mul(e3[:, 0, lo:lo + P], e3[:, 0, lo:lo + P], mask_sb[:])
                    nc.vector.tensor_mul(e3[:, 1, lo:lo + P], e3[:, 1, lo:lo + P], mask_sb[:])
                return (m, lo, e3)

            def _emit_pv(pp, psh, nm, m, lo, e3):
                for h in range(2):
                    hh = 2 * pp + h
                    nc.tensor.matmul(
                        psh[h][0:DK + 1, lo:],
                        v_sb[m][:, hh * (DK + 1): (hh + 1) * (DK + 1)],
                        e3[:, h, lo:],
                        start=(m == 0),
                        stop=(m == nm - 1),
                    )

            def finish_block(pp, j, psh):
                # denominator rows first (they gate the batched reciprocal),
                # then the wider hu copies
                for h in range(2):
                    key = (pp, j, h)
                    if j < nb - 1:
                        row = len(batchA)
                        batchA.append(key)
                        stmp = wpool.tile([1, NQ], f32, tag="stmp", bufs=3)
                        nc.vector.tensor_copy(stmp[:], psh[h][DK:DK + 1, :])
                        nc.sync.dma_start(sumsA[row:row + 1, :], stmp[:])
                    else:
                        row = 32 * len(batchB)
                        batchB.append(key)
                        nc.vector.tensor_copy(sumsB[row:row + 1, :], psh[h][DK:DK + 1, :])
                for h in range(2):
                    key = (pp, j, h)
                    hu[key] = hupool.tile([DK, NQ], bf16, tag="hu", name=f"hu_{pp}_{j}_{h}")
                    nc.vector.tensor_copy(hu[key][:], psh[h][0:DK, :])

            released_x = False
            for j in range(nb):
                filler = []
                if j > 0:
                    # this block's own later v tiles (needed from m = dpb*j)
                    for tk in range(dpb * j, min(dpb * (j + 1), tt)):
                        filler.append(lambda tk=tk: emit_v(tk))
                if j == 0:
                    # kq(pp, n) must complete before block n starts
                    for pp in range(1, npair):
                        filler.append(lambda pp=pp: emit_kq(pp, 0, (0,)))
                        filler.append(lambda pp=pp: emit_kq(pp, 0, (1,)))
                if j < nb - 1:
                    for pp in range(npair):
                        filler.append(lambda pp=pp, n=j + 1: emit_kq(pp, n, (0,)))
                        filler.append(lambda pp=pp, n=j + 1: emit_kq(pp, n, (1,)))
                if j == nb - 1:
                    if nslotsA:
                        filler.append(lambda: emit_normA_recip(nslotsA))
                        # per q-block: its 4 normalization slots, then the
                        # output-projection token blocks they unlock
                        for b in range(nb - 1):
                            for i in range(4 * b, 4 * b + 4):
                                filler.append(lambda i=i: emit_normA_slot(i, nslotsA))
                            for tk in range(dpb * b, dpb * (b + 1)):
                                filler.append(lambda tk=tk: emit_oproj(tk))
                nm = dpb * (j + 1)
                nsteps = npair * nm
                fstate = [0, 0, nsteps]  # steps done, fillers emitted, total
                for pp in range(npair):
                    psh = [psH.tile([P, NQ], f32, tag="h", name=f"psh_{pp}_{j}_{h}") for h in range(2)]
                    pending = None
                    for m in range(nm):
                        step = emit_step(pp, j, m)
                        if pending is not None:
                            _emit_pv(pp, psh, nm, *pending)
                        pending = step
                        fstate[0] += 1
                        while fstate[1] < len(filler) and fstate[1] < fstate[0] * len(filler) // max(fstate[2], 1):
                            filler[fstate[1]]()
                            fstate[1] += 1
                    _emit_pv(pp, psh, nm, *pending)
                    finish_block(pp, j, psh)
                    if j == nb - 1 and pp < npair - 1:
                        # queue this pair's normalization as filler so it
                        # drips into the next pair's steps
                        filler.append(lambda pp=pp: emit_normB(batchB[2 * pp:2 * pp + 2], 2 * pp))
                while fstate[1] < len(filler):
                    filler[fstate[1]]()
                    fstate[1] += 1
                if j >= nb - 2 and not released_x:
                    released_x = True
                    xpool.release()

            # ---- tail: last pair's normalization + last block's output
            # projection, software-pipelined 4 psum buffers deep. The first
            # pair-0 half-accumulations issue before the normalization chain
            # so the PE stays busy (and HAM-warm) while it resolves.
            units = [(tk, ob) for tk in range(dpb * (nb - 1), tt) for ob in range(d // obw)]
            psys = {}

            def tail_p0(u):
                tk, ob = units[u]
                pl, tg, w = ((psF, "acc", obw), (psH, "h", NQ))[u % 2]
                psy = pl.tile([P, w], f32, tag=tg, name=f"yt_{u}")
                nc.tensor.matmul(
                    psy[:, 0:obw],
                    hT[0][nb - 1][:, (tk % dpb) * P:(tk % dpb + 1) * P],
                    wo_sb[:, ob * obw:(ob + 1) * obw],
                    start=True, stop=False,
                )
                psys[u] = psy

            def tail_p1(u):
                tk, ob = units[u]
                psy = psys.pop(u)
                nc.tensor.matmul(
                    psy[:, 0:obw],
                    hT[1][nb - 1][:, (tk % dpb) * P:(tk % dpb + 1) * P],
                    wo_sb[:, d + ob * obw: d + (ob + 1) * obw],
                    start=False, stop=True,
                )
                ysb = wpool.tile([P, obw], bf16, tag="ysb", bufs=4)
                if u % 2:
                    nc.scalar.copy(ysb[:], psy[:, 0:obw])
                else:
                    nc.vector.tensor_copy(ysb[:], psy[:, 0:obw])
                nc.sync.dma_start(y[tk * P:(tk + 1) * P, ob * obw:(ob + 1) * obw], ysb[:])

            for u in range(4):
                tail_p0(u)
            emit_normB(batchB[2 * (npair - 1):2 * (npair - 1) + 2], 2 * (npair - 1), ps_pool=psS)
            for u in range(4):
                tail_p1(u)
            for u in range(4, len(units)):
                tail_p0(u)
            for u in range(4, len(units)):
                tail_p1(u)

    nc.compile()
    return nc


def _get_nc():
    if "nc" not in _CACHE:
        _CACHE["nc"] = _build_nc()
    return _CACHE["nc"]


def _emat_np(nrows):
    import ml_dtypes
    e = np.zeros((nrows, nrows * DK), ml_dtypes.bfloat16)
    for i in range(nrows):
        e[i, i * DK:(i + 1) * DK] = 1.0
    return e


def _masks_np():
    import ml_dtypes
    kk = np.arange(P)[:, None]
    qq = np.arange(P)[None, :]
    return (kk <= qq).astype(ml_dtypes.bfloat16)


def _emat_rows(t=T, od=OD):
    nb = t // NQ
    npair = od // P
    return max(2 * npair * (nb - 1), 1)


def _ematB_np(t=T, od=OD):
    nslotsB = 2 * (od // P)
    nrowsB = 32 * (nslotsB - 1) + 1
    e = np.zeros((nrowsB, nslotsB * DK), np.float32)
    for i in range(nslotsB):
        e[32 * i, i * DK:(i + 1) * DK] = 1.0
    return e


def make_in_maps(x, Wq, Wk, Wv, Wo):
    import ml_dtypes

    bf = ml_dtypes.bfloat16
    x = np.asarray(x, np.float32)
    msk = _masks_np()
    emat = _emat_np(_emat_rows())
    in_maps = []
    for c in range(N_CORES):
        b, g = c // (N_CORES // 2), c % (N_CORES // 2)
        hs = slice(OD * g, OD * (g + 1))
        in_maps.append({
            "xT": np.ascontiguousarray(x[b].T).astype(bf),
            "wqT": np.ascontiguousarray(np.asarray(Wq, np.float32)[hs, :].T).astype(bf),
            "wkT": np.ascontiguousarray(np.asarray(Wk, np.float32)[hs, :].T).astype(bf),
            "wvT": np.ascontiguousarray(np.asarray(Wv, np.float32)[hs, :].T).astype(bf),
            "woT": np.ascontiguousarray(np.asarray(Wo, np.float32)[:, hs].T).astype(bf),
            "masks": msk,
            "emat": emat,
            "ematB": _ematB_np(),
        })
    return in_maps


def combine_outputs(results):
    ng = N_CORES // 2
    out = np.empty((2, T, D), np.float32)
    for b in range(2):
        acc = results[b * ng]["y"].astype(np.float32)
        for g in range(1, ng):
            acc = acc + results[b * ng + g]["y"].astype(np.float32)
        out[b] = acc
    return out


def kernel(x, Wq, Wk, Wv, Wo):
    from concourse.bass_utils import run_bass_kernel_spmd

    nc = _get_nc()
    in_maps = make_in_maps(x, Wq, Wk, Wv, Wo)
    res = run_bass_kernel_spmd(nc, in_maps, list(range(N_CORES)))
    return combine_outputs(res.results)


# revision 22
# speedup vs baseline: 1.0046x; 1.0046x over previous
"""Multi-head causal self-attention on 8 Trainium2 NeuronCores.

Sharding: tensor-parallel over heads (4 heads/core) x data-parallel over
batch (B=2): core c -> batch c//4, head-group c%4. Each core computes its
4 heads' attention plus a partial output projection; the host sums the 4
partials per batch element.

Layout strategy (per core):
  - x is fed pre-transposed (xT: [D, T]) so QKV projections produce
    qT/kT ([head_dim, T], head-dim on partitions) and v ([T, head_dim])
    directly, with no on-device transposes anywhere.
  - Startup: the critical-path DMA (wk/wq/xc-block0 in k-chunk order,
    rotated across three issue queues) goes out first so the QKV matmuls
    start as soon as chunk 0 lands and stream behind the DMA; wv / the
    other x blocks / wo / normalization constants follow in need-order.
  - Scores are computed transposed (k on partitions, q on free dim):
    psum[k, q] = kT_tile.T @ qT_block. Two heads run concurrently via
    row-tiled tile_position (dk=64 each) into one 2-bank psum tile, so
    one Exp activation covers the pair (halves ACT call overhead).
  - Softmax skips max-subtraction (scores are bounded well inside fp32
    exp range); exp runs on ScalarE with scale=1/sqrt(dk) folded in.
    Causal masking multiplies only diagonal tiles by a 0/1 mask, one
    head on VectorE and one on GpSimd.
  - P@V uses an M=65 stationary [v_head | ones] so the softmax
    denominators accumulate in psum row 64 of the same matmul.
  - Normalization: denominator rows are bounced via tiny DMAs into
    collector tiles, inverted with the fast Newton-Raphson reciprocal,
    broadcast across partitions by a one-hot-selector matmul, and
    multiplied in.
  - The exp chain paces the attention phase, so the PE stream is kept
    stall-free to hold the HAM clock at 2.4GHz: the m-loop is software
    pipelined (scores of step m+1 issue before P@V of step m), the two
    head pairs' blocks alternate, and QKV / output-projection matmul
    groups are interleaved into the attention phase as PE filler.
  - Matmul operands are bf16 (1 cycle/column on the PE; fp32r costs 2).
    Accumulation is fp32 in PSUM; denominators/reciprocals stay fp32.
  - y partials leave as bf16 (halves output DMA); the host sums in fp32.
"""

import sys

for _p in ("/opt/trn_rl_repo",):
    if _p not in sys.path:
        sys.path.append(_p)

import numpy as np

P = 128
T = 2048
D = 1024
OD = 256  # output dims per core = 4 heads x 64
DK = 64
NQ = 512  # q-block (psum free size)
N_CORES = 8

_CACHE = {}


def _build_nc(t=T, d=D, od=OD):
    import concourse.bass as bass
    import concourse.tile as tile
    from concourse import bacc, mybir

    f32 = mybir.dt.float32
    f32r = mybir.dt.float32r
    bf16 = mybir.dt.bfloat16

    kt = d // P        # k-tiles over d_model
    tt = t // P        # token tiles
    nb = t // NQ       # q blocks
    npair = od // P    # head pairs (2 heads per 128 partitions)
    dpb = NQ // P      # diagonal k-tiles per q block
    nh = od // DK      # heads per core

    nslotsA = 2 * npair * (nb - 1)  # (pair, j, head) slots with j < nb-1
    nslotsB = 2 * npair             # slots with j == nb-1
    nrows = max(nslotsA, 1)
    nrowsB = 32 * (nslotsB - 1) + 1  # batch-B rows live at partitions 32*i

    nc = bacc.Bacc("TRN2", target_bir_lowering=False, debug=False)

    xT = nc.dram_tensor("xT", [d, t], bf16, kind="ExternalInput")
    wqT = nc.dram_tensor("wqT", [d, od], bf16, kind="ExternalInput")
    wkT = nc.dram_tensor("wkT", [d, od], bf16, kind="ExternalInput")
    wvT = nc.dram_tensor("wvT", [d, od], bf16, kind="ExternalInput")
    woT = nc.dram_tensor("woT", [od, d], bf16, kind="ExternalInput")
    masks = nc.dram_tensor("masks", [P, P], bf16, kind="ExternalInput")
    emat = nc.dram_tensor("emat", [nrows, nrows * DK], bf16, kind="ExternalInput")
    ematB = nc.dram_tensor("ematB", [nrowsB, nslotsB * DK], f32r, kind="ExternalInput")
    y = nc.dram_tensor("y", [t, d], bf16, kind="ExternalOutput")

    Exp = mybir.ActivationFunctionType.Exp
    scale = 1.0 / float(np.sqrt(DK))

    with tile.TileContext(nc) as tc:
        with (
            tc.tile_pool(name="const", bufs=1) as cpool,
            tc.tile_pool(name="qk", bufs=2 * npair * nb) as qkpool,
            tc.tile_pool(name="vp", bufs=tt) as vpool,
            tc.tile_pool(name="ht", bufs=npair * nb) as hpool,
            tc.tile_pool(name="hu", bufs=2 * npair * nb) as hupool,
            tc.tile_pool(name="work", bufs=8) as wpool,
            tc.tile_pool(name="psS", bufs=2, space="PSUM") as psS,
            tc.tile_pool(name="psH", bufs=2, space="PSUM") as psH,
            tc.tile_pool(name="psF", bufs=2, space="PSUM") as psF,
        ):
            # ---- constant tiles (DMAs for the late-needed ones issue last) ----
            wo_sb = cpool.tile([P, npair * d], bf16, tag="wo")
            mask_sb = cpool.tile([P, P], bf16, tag="mask")
            emat_sb = cpool.tile([nrows, nrows * DK], bf16, tag="emat")
            ematB_sb = cpool.tile([nrowsB, nslotsB * DK], f32r, tag="ematB")

            # x and the QKV weights live in a scoped pool released after the
            # projections, freeing space for the attention phase.
            xpool = tc.alloc_tile_pool(name="xp", bufs=1)
            xc = [xpool.tile([P, kt * NQ], bf16, tag=f"xc{c}", name=f"xc_{c}") for c in range(nb)]
            wq_sb = xpool.tile([P, kt * od], bf16, tag="wq")
            wk_sb = xpool.tile([P, kt * od], bf16, tag="wk")
            wv_sb = xpool.tile([P, kt * od], bf16, tag="wv")

            _eng = [nc.sync, nc.gpsimd, nc.scalar]
            _ei = [0]

            def _issue(dst, src):
                _eng[_ei[0] % 3].dma_start(dst, src)
                _ei[0] += 1

            xTv = xT.rearrange("(k p) q -> p k q", p=P)
            wqv = wqT.rearrange("(k p) o -> p k o", p=P)
            wkv = wkT.rearrange("(k p) o -> p k o", p=P)
            wvv = wvT.rearrange("(k p) o -> p k o", p=P)
            xdst = [xc[c][:].rearrange("p (k q) -> p k q", q=NQ) for c in range(nb)]
            wqd = wq_sb[:].rearrange("p (k o) -> p k o", o=od)
            wkd = wk_sb[:].rearrange("p (k o) -> p k o", o=od)
            wvd = wv_sb[:].rearrange("p (k o) -> p k o", o=od)

            # mask first (needed at the very first diagonal step; tiny)
            nc.sync.dma_start(mask_sb[:], masks[:])
            # critical path: wk/wq/xc0 k-chunks, earliest k first, spread so
            # every queue carries a mix (the first matmul waits only on the
            # first chunks of each)
            for i, kp in enumerate(range(0, kt, 2)):
                _eng[(i + 0) % 3].dma_start(wkd[:, kp:kp + 2], wkv[:, kp:kp + 2])
                _eng[(i + 1) % 3].dma_start(wqd[:, kp:kp + 2], wqv[:, kp:kp + 2])
                _eng[(i + 2) % 3].dma_start(xdst[0][:, kp:kp + 2], xTv[:, kp:kp + 2, 0:NQ])
            # wv next (v-projection warm-up), then the remaining x blocks,
            # then output-side constants
            for kp in range(0, kt, 4):
                _issue(wvd[:, kp:kp + 4], wvv[:, kp:kp + 4])
            for c in range(1, nb):
                for h0 in range(0, kt, 4):
                    _issue(xdst[c][:, h0:h0 + 4], xTv[:, h0:h0 + 4, c * NQ:(c + 1) * NQ])
            for pp in range(npair):
                _issue(wo_sb[:, pp * d:(pp + 1) * d], woT[pp * P:(pp + 1) * P, :])
            _issue(emat_sb[:], emat[:])
            _issue(ematB_sb[:], ematB[:])

            # ---- persistent tiles ----
            qT = [[qkpool.tile([P, NQ], bf16, tag="qT", name=f"qT_{pp}_{n}") for n in range(nb)] for pp in range(npair)]
            kT = [[qkpool.tile([P, NQ], bf16, tag="kT", name=f"kT_{pp}_{n}") for n in range(nb)] for pp in range(npair)]
            v_sb = [vpool.tile([P, nh * (DK + 1)], bf16, tag="v", name=f"v_{tk}") for tk in range(tt)]
            hT = [[hpool.tile([P, NQ], bf16, tag="hT", name=f"hT_{pp}_{n}") for n in range(nb)] for pp in range(npair)]
            hu = {}

            sumsA = wpool.tile([max(nslotsA, 1), NQ], f32, tag="sumsA", bufs=1)
            sumsB = wpool.tile([nrowsB, NQ], f32, tag="sumsB", bufs=1)
            nc.vector.memset(sumsB[:], 1.0)
            batchA = []  # (pp, j, h) in collector-row order
            batchB = []

            # ---- emit helpers ----
            def emit_kq(pp, n, which=(0, 1)):
                for idx, (dst, w_sb) in enumerate(((kT, wk_sb), (qT, wq_sb))):
                    if idx not in which:
                        continue
                    ps = psF.tile([P, NQ], f32, tag="acc", name=f"kqps_{pp}_{n}_{idx}")
                    for k in range(kt):
                        nc.tensor.matmul(
                            ps[:],
                            w_sb[:, k * od + pp * P: k * od + (pp + 1) * P],
                            xc[n][:, k * NQ:(k + 1) * NQ],
                            start=(k == 0),
                            stop=(k == kt - 1),
                        )
                    nc.vector.tensor_copy(dst[pp][n][:], ps[:])

            def emit_v(tk):
                # each head's 64 v-columns are followed by a ones column so
                # the P@V matmul also accumulates the softmax denominator
                vv = v_sb[tk][:].rearrange("p (h c) -> p h c", c=DK + 1)
                nc.vector.memset(v_sb[tk][:], 1.0)
                ps = psF.tile([P, od], f32, tag="acc", name=f"vps_{tk}")
                for k in range(kt):
                    nc.tensor.matmul(
                        ps[:],
                        xc[tk // dpb][:, k * NQ + (tk % dpb) * P: k * NQ + (tk % dpb + 1) * P],
                        wv_sb[:, k * od:(k + 1) * od],
                        start=(k == 0),
                        stop=(k == kt - 1),
                    )
                nc.vector.tensor_copy(
                    vv[:, :, 0:DK],
                    ps[:].rearrange("p (h c) -> p h c", c=DK),
                )

            invA_holder = {}

            def emit_normA_recip(nslots):
                invf = wpool.tile([nslots, NQ], f32, tag="invAf", bufs=1, name="inv_Af")
                nc.vector.reciprocal_approx_fast(invf[:], sumsA[0:nslots, :])
                inv = wpool.tile([nslots, NQ], bf16, tag="invA", bufs=1, name="inv_A")
                nc.vector.tensor_copy(inv[:], invf[:])
                invA_holder["inv"] = inv

            def emit_normA_slot(i, nslots):
                pp, j, h = batchA[i]
                inv = invA_holder["inv"]
                psb = psF.tile([DK, NQ], f32, tag="acc", name=f"psb_A_{i}")
                nc.tensor.matmul(psb[:], emat_sb[0:nslots, i * DK:(i + 1) * DK], inv[:], start=True, stop=True)
                nc.vector.tensor_mul(hT[pp][j][h * DK:(h + 1) * DK, :], hu[(pp, j, h)][:], psb[:])

            def emit_normB(part, lo_i, ps_pool=None):
                rows = 32 * (len(part) - 1) + 1
                invB = wpool.tile([rows, NQ], f32r, tag="invB", bufs=2, name=f"invB_{lo_i}")
                with nc.allow_low_precision(reason="f32r shares f32 bits"):
                    nc.vector.reciprocal(invB[:], sumsB[32 * lo_i:32 * lo_i + rows, :])
                for i, (pp, j, h) in enumerate(part):
                    psb = (ps_pool or psF).tile([DK, NQ], f32, tag="acc", name=f"psbB_{lo_i}_{i}")
                    nc.tensor.matmul(psb[:], ematB_sb[0:rows, i * DK:(i + 1) * DK], invB[:], start=True, stop=True)
                    nc.vector.tensor_mul(hT[pp][j][h * DK:(h + 1) * DK, :], hu[(pp, j, h)][:], psb[:])

            obw = min(NQ, d)

            def emit_oproj(tk, use_act=False):
                for ob in range(d // obw):
                    psy = psF.tile([P, obw], f32, tag="acc", name=f"yps_{tk}_{ob}")
                    for pp in range(npair):
                        nc.tensor.matmul(
                            psy[:],
                            hT[pp][tk // dpb][:, (tk % dpb) * P:(tk % dpb + 1) * P],
                            wo_sb[:, pp * d + ob * obw: pp * d + (ob + 1) * obw],
                            start=(pp == 0),
                            stop=(pp == npair - 1),
                        )
                    ysb = wpool.tile([P, obw], bf16, tag="ysb", bufs=4)
                    if use_act:
                        nc.scalar.copy(ysb[:], psy[:])
                    else:
                        nc.vector.tensor_copy(ysb[:], psy[:])
                    nc.sync.dma_start(y[tk * P:(tk + 1) * P, ob * obw:(ob + 1) * obw], ysb[:])

            # ---- warm-up: just enough for (pair 0, block 0) ----
            emit_kq(0, 0)
            for tk in range(min(dpb, tt)):
                emit_v(tk)

            # ---- attention m-step: both heads' scores land in one 2-bank
            # psum tile so a single Exp covers the pair; only the 128-wide
            # diagonal boundary needs the causal mask ----
            def emit_step(pp, j, m):
                dlt = m - dpb * j
                lo = max(dlt, 0) * P  # first live q column of this k-tile
                pss = psS.tile([P, 2 * NQ], f32, tag="acc", name=f"pss_{pp}_{j}_{m}")
                ps3 = pss[:].rearrange("p (h q) -> p h q", q=NQ)
                for h in range(2):
                    nc.tensor.matmul(
                        ps3[:, h, lo:],
                        kT[pp][m // dpb][h * DK:(h + 1) * DK, (m % dpb) * P:(m % dpb + 1) * P],
                        qT[pp][j][h * DK:(h + 1) * DK, lo:],
                        start=True,
                        stop=True,
                        tile_position=(h * DK, 0),
                    )
                e = wpool.tile([P, 2 * NQ], bf16, tag="exp", bufs=12)
                e3 = e[:].rearrange("p (h q) -> p h q", q=NQ)
                nc.scalar.activation(e3[:, :, lo:], ps3[:, :, lo:], Exp, bias=0.0, scale=scale)
                if dlt >= 0:
                    nc.vector.tensor_mul(e3[:, 0, lo:lo + P], e3[:, 0, lo:lo + P], mask_sb[:])
                    nc.vector.tensor_mul(e3[:, 1, lo:lo + P], e3[:, 1, lo:lo + P], mask_sb[:])
                return (m, lo, e3)

            def _emit_pv(pp, psh, nm, m, lo, e3):
                for h in range(2):
                    hh = 2 * pp + h
                    nc.tensor.matmul(
                        psh[h][0:DK + 1, lo:],
                        v_sb[m][:, hh * (DK + 1): (hh + 1) * (DK + 1)],
                        e3[:, h, lo:],
                        start=(m == 0),
                        stop=(m == nm - 1),
                    )

            def finish_block(pp, j, psh):
                # denominator rows first (they gate the batched reciprocal),
                # then the wider hu copies
                for h in range(2):
                    key = (pp, j, h)
                    if j < nb - 1:
                        row = len(batchA)
                        batchA.append(key)
                        stmp = wpool.tile([1, NQ], f32, tag="stmp", bufs=3)
                        nc.vector.tensor_copy(stmp[:], psh[h][DK:DK + 1, :])
                        nc.sync.dma_start(sumsA[row:row + 1, :], stmp[:])
                    else:
                        row = 32 * len(batchB)
                        batchB.append(key)
                        nc.vector.tensor_copy(sumsB[row:row + 1, :], psh[h][DK:DK + 1, :])
                for h in range(2):
                    key = (pp, j, h)
                    hu[key] = hupool.tile([DK, NQ], bf16, tag="hu", name=f"hu_{pp}_{j}_{h}")
                    nc.vector.tensor_copy(hu[key][:], psh[h][0:DK, :])

            released_x = False
            for j in range(nb):
                filler = []
                if j > 0:
                    # this block's own later v tiles (needed from m = dpb*j)
                    for tk in range(dpb * j, min(dpb * (j + 1), tt)):
                        filler.append(lambda tk=tk: emit_v(tk))
                if j == 0:
                    # kq(pp, n) must complete before block n starts
                    for pp in range(1, npair):
                        filler.append(lambda pp=pp: emit_kq(pp, 0, (0,)))
                        filler.append(lambda pp=pp: emit_kq(pp, 0, (1,)))
                if j < nb - 1:
                    for pp in range(npair):
                        filler.append(lambda pp=pp, n=j + 1: emit_kq(pp, n, (0,)))
                        filler.append(lambda pp=pp, n=j + 1: emit_kq(pp, n, (1,)))
                if j == nb - 1:
                    if nslotsA:
                        filler.append(lambda: emit_normA_recip(nslotsA))
                        # per q-block: its 4 normalization slots, then the
                        # output-projection token blocks they unlock
                        for b in range(nb - 1):
                            for i in range(4 * b, 4 * b + 4):
                                filler.append(lambda i=i: emit_normA_slot(i, nslotsA))
                            for tk in range(dpb * b, dpb * (b + 1)):
                                filler.append(lambda tk=tk: emit_oproj(tk))
                nm = dpb * (j + 1)
                nsteps = npair * nm
                fstate = [0, 0, nsteps]  # steps done, fillers emitted, total
                for pp in range(npair):
                    psh = [psH.tile([P, NQ], f32, tag="h", name=f"psh_{pp}_{j}_{h}") for h in range(2)]
                    pending = None
                    for m in range(nm):
                        step = emit_step(pp, j, m)
                        if pending is not None:
                            _emit_pv(pp, psh, nm, *pending)
                        pending = step
                        fstate[0] += 1
                        while fstate[1] < len(filler) and fstate[1] < fstate[0] * len(filler) // max(fstate[2], 1):
                            filler[fstate[1]]()
                            fstate[1] += 1
                    _emit_pv(pp, psh, nm, *pending)
                    finish_block(pp, j, psh)
                    if j == nb - 1 and pp < npair - 1:
                        # queue this pair's normalization as filler so it
                        # drips into the next pair's steps
                        filler.append(lambda pp=pp: emit_normB(batchB[2 * pp:2 * pp + 2], 2 * pp))
                while fstate[1] < len(filler):
                    filler[fstate[1]]()
                    fstate[1] += 1
                if j >= nb - 2 and not released_x:
                    released_x = True
                    xpool.release()

            # ---- tail: last pair's normalization + last block's output
            # projection, software-pipelined 4 psum buffers deep. The first
            # pair-0 half-accumulations issue before the normalization chain
            # so the PE stays busy (and HAM-warm) while it resolves.
            units = [(tk, ob) for tk in range(dpb * (nb - 1), tt) for ob in range(d // obw)]
            psys = {}

            def tail_p0(u):
                tk, ob = units[u]
                pl, tg, w = ((psF, "acc", obw), (psH, "h", NQ))[u % 2]
                psy = pl.tile([P, w], f32, tag=tg, name=f"yt_{u}")
                nc.tensor.matmul(
                    psy[:, 0:obw],
                    hT[0][nb - 1][:, (tk % dpb) * P:(tk % dpb + 1) * P],
                    wo_sb[:, ob * obw:(ob + 1) * obw],
                    start=True, stop=False,
                )
                psys[u] = psy

            def tail_p1(u):
                tk, ob = units[u]
                psy = psys.pop(u)
                nc.tensor.matmul(
                    psy[:, 0:obw],
                    hT[1][nb - 1][:, (tk % dpb) * P:(tk % dpb + 1) * P],
                    wo_sb[:, d + ob * obw: d + (ob + 1) * obw],
                    start=False, stop=True,
                )
                ysb = wpool.tile([P, obw], bf16, tag="ysb", bufs=4)
                if u % 2:
                    nc.scalar.copy(ysb[:], psy[:, 0:obw])
                else:
                    nc.vector.tensor_copy(ysb[:], psy[:, 0:obw])
                nc.sync.dma_start(y[tk * P:(tk + 1) * P, ob * obw:(ob + 1) * obw], ysb[:])

            for u in range(4):
                tail_p0(u)
            emit_normB(batchB[2 * (npair - 1):2 * (npair - 1) + 2], 2 * (npair - 1), ps_pool=psS)
            for u in range(4):
                tail_p1(u)
            for u in range(4, len(units)):
                tail_p0(u)
            for u in range(4, len(units)):
                tail_p1(u)

    nc.compile()
    return nc


def _get_nc():
    if "nc" not in _CACHE:
        _CACHE["nc"] = _build_nc()
    return _CACHE["nc"]


def _emat_np(nrows):
    import ml_dtypes
    e = np.zeros((nrows, nrows * DK), ml_dtypes.bfloat16)
    for i in range(nrows):
        e[i, i * DK:(i + 1) * DK] = 1.0
    return e


def _masks_np():
    import ml_dtypes
    kk = np.arange(P)[:, None]
    qq = np.arange(P)[None, :]
    return (kk <= qq).astype(ml_dtypes.bfloat16)


def _emat_rows(t=T, od=OD):
    nb = t // NQ
    npair = od // P
    return max(2 * npair * (nb - 1), 1)


def _ematB_np(t=T, od=OD):
    nslotsB = 2 * (od // P)
    nrowsB = 32 * (nslotsB - 1) + 1
    e = np.zeros((nrowsB, nslotsB * DK), np.float32)
    for i in range(nslotsB):
        e[32 * i, i * DK:(i + 1) * DK] = 1.0
    return e


def make_in_maps(x, Wq, Wk, Wv, Wo):
    import ml_dtypes

    bf = ml_dtypes.bfloat16
    x = np.asarray(x, np.float32)
    msk = _masks_np()
    emat = _emat_np(_emat_rows())
    in_maps = []
    for c in range(N_CORES):
        b, g = c // (N_CORES // 2), c % (N_CORES // 2)
        hs = slice(OD * g, OD * (g + 1))
        in_maps.append({
            "xT": np.ascontiguousarray(x[b].T).astype(bf),
            "wqT": np.ascontiguousarray(np.asarray(Wq, np.float32)[hs, :].T).astype(bf),
            "wkT": np.ascontiguousarray(np.asarray(Wk, np.float32)[hs, :].T).astype(bf),
            "wvT": np.ascontiguousarray(np.asarray(Wv, np.float32)[hs, :].T).astype(bf),
            "woT": np.ascontiguousarray(np.asarray(Wo, np.float32)[:, hs].T).astype(bf),
            "masks": msk,
            "emat": emat,
            "ematB": _ematB_np(),
        })
    return in_maps


def combine_outputs(results):
    ng = N_CORES // 2
    out = np.empty((2, T, D), np.float32)
    for b in range(2):
        acc = results[b * ng]["y"].astype(np.float32)
        for g in range(1, ng):
            acc = acc + results[b * ng + g]["y"].astype(np.float32)
        out[b] = acc
    return out


def kernel(x, Wq, Wk, Wv, Wo):
    from concourse.bass_utils import run_bass_kernel_spmd

    nc = _get_nc()
    in_maps = make_in_maps(x, Wq, Wk, Wv, Wo)
    res = run_bass_kernel_spmd(nc, in_maps, list(range(N_CORES)))
    return combine_outputs(res.results)


# revision 26
# speedup vs baseline: 1.0078x; 1.0031x over previous
"""Multi-head causal self-attention on 8 Trainium2 NeuronCores.

Sharding: tensor-parallel over heads (4 heads/core) x data-parallel over
batch (B=2): core c -> batch c//4, head-group c%4. Each core computes its
4 heads' attention plus a partial output projection; the host sums the 4
partials per batch element.

Layout strategy (per core):
  - x is fed pre-transposed (xT: [D, T]) so QKV projections produce
    qT/kT ([head_dim, T], head-dim on partitions) and v ([T, head_dim])
    directly, with no on-device transposes anywhere.
  - Startup: the critical-path DMA (wk/wq/xc-block0 in k-chunk order,
    rotated across three issue queues) goes out first so the QKV matmuls
    start as soon as chunk 0 lands and stream behind the DMA; wv / the
    other x blocks / wo / normalization constants follow in need-order.
  - Scores are computed transposed (k on partitions, q on free dim):
    psum[k, q] = kT_tile.T @ qT_block. Two heads run concurrently via
    row-tiled tile_position (dk=64 each) into one 2-bank psum tile, so
    one Exp activation covers the pair (halves ACT call overhead).
  - Softmax skips max-subtraction (scores are bounded well inside fp32
    exp range); exp runs on ScalarE with scale=1/sqrt(dk) folded in.
    Causal masking multiplies only diagonal tiles by a 0/1 mask, one
    head on VectorE and one on GpSimd.
  - P@V uses an M=65 stationary [v_head | ones] so the softmax
    denominators accumulate in psum row 64 of the same matmul.
  - Normalization: denominator rows are bounced via tiny DMAs into
    collector tiles, inverted with the fast Newton-Raphson reciprocal,
    broadcast across partitions by a one-hot-selector matmul, and
    multiplied in.
  - The exp chain paces the attention phase, so the PE stream is kept
    stall-free to hold the HAM clock at 2.4GHz: the m-loop is software
    pipelined (scores of step m+1 issue before P@V of step m), the two
    head pairs' blocks alternate, and QKV / output-projection matmul
    groups are interleaved into the attention phase as PE filler.
  - Matmul operands are bf16 (1 cycle/column on the PE; fp32r costs 2).
    Accumulation is fp32 in PSUM; denominators/reciprocals stay fp32.
  - y partials leave as bf16 (halves output DMA); the host sums in fp32.
"""

import sys

for _p in ("/opt/trn_rl_repo",):
    if _p not in sys.path:
        sys.path.append(_p)

import numpy as np

P = 128
T = 2048
D = 1024
OD = 256  # output dims per core = 4 heads x 64
DK = 64
NQ = 512  # q-block (psum free size)
N_CORES = 8

_CACHE = {}


def _build_nc(t=T, d=D, od=OD):
    import concourse.bass as bass
    import concourse.tile as tile
    from concourse import bacc, mybir

    f32 = mybir.dt.float32
    f32r = mybir.dt.float32r
    bf16 = mybir.dt.bfloat16

    kt = d // P        # k-tiles over d_model
    tt = t // P        # token tiles
    nb = t // NQ       # q blocks
    npair = od // P    # head pairs (2 heads per 128 partitions)
    dpb = NQ // P      # diagonal k-tiles per q block
    nh = od // DK      # heads per core

    nslotsA = 2 * npair * (nb - 1)  # (pair, j, head) slots with j < nb-1
    nslotsB = 2 * npair             # slots with j == nb-1
    nrows = max(nslotsA, 1)
    nrowsB = 32 * (nslotsB - 1) + 1  # batch-B rows live at partitions 32*i

    nc = bacc.Bacc("TRN2", target_bir_lowering=False, debug=False)

    xT = nc.dram_tensor("xT", [d, t], bf16, kind="ExternalInput")
    wqT = nc.dram_tensor("wqT", [d, od], bf16, kind="ExternalInput")
    wkT = nc.dram_tensor("wkT", [d, od], bf16, kind="ExternalInput")
    wvT = nc.dram_tensor("wvT", [d, od], bf16, kind="ExternalInput")
    woT = nc.dram_tensor("woT", [od, d], bf16, kind="ExternalInput")
    masks = nc.dram_tensor("masks", [P, P], bf16, kind="ExternalInput")
    emat = nc.dram_tensor("emat", [nrows, nrows * DK], bf16, kind="ExternalInput")
    ematB = nc.dram_tensor("ematB", [nrowsB, nslotsB * DK], f32r, kind="ExternalInput")
    y = nc.dram_tensor("y", [t, d], bf16, kind="ExternalOutput")

    Exp = mybir.ActivationFunctionType.Exp
    scale = 1.0 / float(np.sqrt(DK))

    with tile.TileContext(nc) as tc:
        with (
            tc.tile_pool(name="const", bufs=1) as cpool,
            tc.tile_pool(name="qk", bufs=2 * npair * nb) as qkpool,
            tc.tile_pool(name="vp", bufs=tt) as vpool,
            tc.tile_pool(name="ht", bufs=npair * nb) as hpool,
            tc.tile_pool(name="hu", bufs=2 * npair * nb) as hupool,
            tc.tile_pool(name="work", bufs=8) as wpool,
            tc.tile_pool(name="psS", bufs=2, space="PSUM") as psS,
            tc.tile_pool(name="psH", bufs=2, space="PSUM") as psH,
            tc.tile_pool(name="psF", bufs=2, space="PSUM") as psF,
        ):
            # ---- constant tiles (DMAs for the late-needed ones issue last) ----
            wo_sb = cpool.tile([P, npair * d], bf16, tag="wo")
            mask_sb = cpool.tile([P, P], bf16, tag="mask")
            emat_sb = cpool.tile([nrows, nrows * DK], bf16, tag="emat")
            ematB_sb = cpool.tile([nrowsB, nslotsB * DK], f32r, tag="ematB")

            # x and the QKV weights live in a scoped pool released after the
            # projections, freeing space for the attention phase.
            xpool = tc.alloc_tile_pool(name="xp", bufs=1)
            xc = [xpool.tile([P, kt * NQ], bf16, tag=f"xc{c}", name=f"xc_{c}") for c in range(nb)]
            wq_sb = xpool.tile([P, kt * od], bf16, tag="wq")
            wk_sb = xpool.tile([P, kt * od], bf16, tag="wk")
            wv_sb = xpool.tile([P, kt * od], bf16, tag="wv")

            _eng = [nc.sync, nc.gpsimd, nc.scalar]
            _ei = [0]

            def _issue(dst, src):
                _eng[_ei[0] % 3].dma_start(dst, src)
                _ei[0] += 1

            xTv = xT.rearrange("(k p) q -> p k q", p=P)
            wqv = wqT.rearrange("(k p) o -> p k o", p=P)
            wkv = wkT.rearrange("(k p) o -> p k o", p=P)
            wvv = wvT.rearrange("(k p) o -> p k o", p=P)
            xdst = [xc[c][:].rearrange("p (k q) -> p k q", q=NQ) for c in range(nb)]
            wqd = wq_sb[:].rearrange("p (k o) -> p k o", o=od)
            wkd = wk_sb[:].rearrange("p (k o) -> p k o", o=od)
            wvd = wv_sb[:].rearrange("p (k o) -> p k o", o=od)

            # mask first (needed at the very first diagonal step; tiny)
            nc.sync.dma_start(mask_sb[:], masks[:])
            # critical path: wk/wq/xc0 k-chunks, earliest k first, spread so
            # every queue carries a mix (the first matmul waits only on the
            # first chunks of each)
            for i, kp in enumerate(range(0, kt, 2)):
                _eng[(i + 0) % 3].dma_start(wkd[:, kp:kp + 2], wkv[:, kp:kp + 2])
                _eng[(i + 1) % 3].dma_start(wqd[:, kp:kp + 2], wqv[:, kp:kp + 2])
                _eng[(i + 2) % 3].dma_start(xdst[0][:, kp:kp + 2], xTv[:, kp:kp + 2, 0:NQ])
            # wv next (v-projection warm-up), then the remaining x blocks,
            # then output-side constants
            for kp in range(0, kt, 4):
                _issue(wvd[:, kp:kp + 4], wvv[:, kp:kp + 4])
            for c in range(1, nb):
                for h0 in range(0, kt, 4):
                    _issue(xdst[c][:, h0:h0 + 4], xTv[:, h0:h0 + 4, c * NQ:(c + 1) * NQ])
            for pp in range(npair):
                _issue(wo_sb[:, pp * d:(pp + 1) * d], woT[pp * P:(pp + 1) * P, :])
            _issue(emat_sb[:], emat[:])
            _issue(ematB_sb[:], ematB[:])

            # ---- persistent tiles ----
            qT = [[qkpool.tile([P, NQ], bf16, tag="qT", name=f"qT_{pp}_{n}") for n in range(nb)] for pp in range(npair)]
            kT = [[qkpool.tile([P, NQ], bf16, tag="kT", name=f"kT_{pp}_{n}") for n in range(nb)] for pp in range(npair)]
            v_sb = [vpool.tile([P, nh * (DK + 1)], bf16, tag="v", name=f"v_{tk}") for tk in range(tt)]
            hT = [[hpool.tile([P, NQ], bf16, tag="hT", name=f"hT_{pp}_{n}") for n in range(nb)] for pp in range(npair)]
            hu = {}

            sumsA = wpool.tile([max(nslotsA, 1), NQ], f32, tag="sumsA", bufs=1)
            sumsB = wpool.tile([nrowsB, NQ], f32, tag="sumsB", bufs=1)
            nc.vector.memset(sumsB[:], 1.0)
            batchA = []  # (pp, j, h) in collector-row order
            batchB = []

            # ---- emit helpers ----
            def emit_kq(pp, n, which=(0, 1)):
                for idx, (dst, w_sb) in enumerate(((kT, wk_sb), (qT, wq_sb))):
                    if idx not in which:
                        continue
                    ps = psF.tile([P, NQ], f32, tag="acc", name=f"kqps_{pp}_{n}_{idx}")
                    for k in range(kt):
                        nc.tensor.matmul(
                            ps[:],
                            w_sb[:, k * od + pp * P: k * od + (pp + 1) * P],
                            xc[n][:, k * NQ:(k + 1) * NQ],
                            start=(k == 0),
                            stop=(k == kt - 1),
                        )
                    nc.vector.tensor_copy(dst[pp][n][:], ps[:])

            def emit_v(tk):
                # each head's 64 v-columns are followed by a ones column so
                # the P@V matmul also accumulates the softmax denominator
                vv = v_sb[tk][:].rearrange("p (h c) -> p h c", c=DK + 1)
                nc.vector.memset(v_sb[tk][:], 1.0)
                ps = psF.tile([P, od], f32, tag="acc", name=f"vps_{tk}")
                for k in range(kt):
                    nc.tensor.matmul(
                        ps[:],
                        xc[tk // dpb][:, k * NQ + (tk % dpb) * P: k * NQ + (tk % dpb + 1) * P],
                        wv_sb[:, k * od:(k + 1) * od],
                        start=(k == 0),
                        stop=(k == kt - 1),
                    )
                nc.vector.tensor_copy(
                    vv[:, :, 0:DK],
                    ps[:].rearrange("p (h c) -> p h c", c=DK),
                )

            invA_holder = {}

            def emit_normA_recip(nslots):
                invf = wpool.tile([nslots, NQ], f32, tag="invAf", bufs=1, name="inv_Af")
                nc.vector.reciprocal_approx_fast(invf[:], sumsA[0:nslots, :])
                inv = wpool.tile([nslots, NQ], bf16, tag="invA", bufs=1, name="inv_A")
                nc.vector.tensor_copy(inv[:], invf[:])
                invA_holder["inv"] = inv

            def emit_normA_slot(i, nslots):
                pp, j, h = batchA[i]
                inv = invA_holder["inv"]
                psb = psF.tile([DK, NQ], f32, tag="acc", name=f"psb_A_{i}")
                nc.tensor.matmul(psb[:], emat_sb[0:nslots, i * DK:(i + 1) * DK], inv[:], start=True, stop=True)
                nc.vector.tensor_mul(hT[pp][j][h * DK:(h + 1) * DK, :], hu[(pp, j, h)][:], psb[:])

            def emit_normB(part, lo_i, ps_pool=None):
                rows = 32 * (len(part) - 1) + 1
                invB = wpool.tile([rows, NQ], f32r, tag="invB", bufs=2, name=f"invB_{lo_i}")
                with nc.allow_low_precision(reason="f32r shares f32 bits"):
                    nc.vector.reciprocal(invB[:], sumsB[32 * lo_i:32 * lo_i + rows, :])
                for i, (pp, j, h) in enumerate(part):
                    psb = (ps_pool or psF).tile([DK, NQ], f32, tag="acc", name=f"psbB_{lo_i}_{i}")
                    nc.tensor.matmul(psb[:], ematB_sb[0:rows, i * DK:(i + 1) * DK], invB[:], start=True, stop=True)
                    nc.vector.tensor_mul(hT[pp][j][h * DK:(h + 1) * DK, :], hu[(pp, j, h)][:], psb[:])

            obw = min(NQ, d)

            def emit_oproj(tk, use_act=False):
                for ob in range(d // obw):
                    psy = psF.tile([P, obw], f32, tag="acc", name=f"yps_{tk}_{ob}")
                    for pp in range(npair):
                        nc.tensor.matmul(
                            psy[:],
                            hT[pp][tk // dpb][:, (tk % dpb) * P:(tk % dpb + 1) * P],
                            wo_sb[:, pp * d + ob * obw: pp * d + (ob + 1) * obw],
                            start=(pp == 0),
                            stop=(pp == npair - 1),
                        )
                    ysb = wpool.tile([P, obw], bf16, tag="ysb", bufs=4)
                    if use_act or ob == 1:
                        nc.scalar.copy(ysb[:], psy[:])
                    else:
                        nc.vector.tensor_copy(ysb[:], psy[:])
                    nc.sync.dma_start(y[tk * P:(tk + 1) * P, ob * obw:(ob + 1) * obw], ysb[:])

            # ---- warm-up: just enough for (pair 0, block 0) ----
            emit_kq(0, 0)
            for tk in range(min(dpb, tt)):
                emit_v(tk)

            # ---- attention m-step: both heads' scores land in one 2-bank
            # psum tile so a single Exp covers the pair; only the 128-wide
            # diagonal boundary needs the causal mask ----
            def emit_step(pp, j, m):
                dlt = m - dpb * j
                lo = max(dlt, 0) * P  # first live q column of this k-tile
                pss = psS.tile([P, 2 * NQ], f32, tag="acc", name=f"pss_{pp}_{j}_{m}")
                ps3 = pss[:].rearrange("p (h q) -> p h q", q=NQ)
                for h in range(2):
                    nc.tensor.matmul(
                        ps3[:, h, lo:],
                        kT[pp][m // dpb][h * DK:(h + 1) * DK, (m % dpb) * P:(m % dpb + 1) * P],
                        qT[pp][j][h * DK:(h + 1) * DK, lo:],
                        start=True,
                        stop=True,
                        tile_position=(h * DK, 0),
                    )
                e = wpool.tile([P, 2 * NQ], bf16, tag="exp", bufs=12)
                e3 = e[:].rearrange("p (h q) -> p h q", q=NQ)
                nc.scalar.activation(e3[:, :, lo:], ps3[:, :, lo:], Exp, bias=0.0, scale=scale)
                if dlt >= 0:
                    nc.vector.tensor_mul(e3[:, 0, lo:lo + P], e3[:, 0, lo:lo + P], mask_sb[:])
                    nc.gpsimd.tensor_mul(e3[:, 1, lo:lo + P], e3[:, 1, lo:lo + P], mask_sb[:])
                return (m, lo, e3)

            def _emit_pv(pp, psh, nm, m, lo, e3):
                for h in range(2):
                    hh = 2 * pp + h
                    nc.tensor.matmul(
                        psh[h][0:DK + 1, lo:],
                        v_sb[m][:, hh * (DK + 1): (hh + 1) * (DK + 1)],
                        e3[:, h, lo:],
                        start=(m == 0),
                        stop=(m == nm - 1),
                    )

            def finish_block(pp, j, psh):
                # denominator rows first (they gate the batched reciprocal),
                # then the wider hu copies
                for h in range(2):
                    key = (pp, j, h)
                    if j < nb - 1:
                        row = len(batchA)
                        batchA.append(key)
                        stmp = wpool.tile([1, NQ], f32, tag="stmp", bufs=3)
                        nc.vector.tensor_copy(stmp[:], psh[h][DK:DK + 1, :])
                        nc.sync.dma_start(sumsA[row:row + 1, :], stmp[:])
                    else:
                        row = 32 * len(batchB)
                        batchB.append(key)
                        nc.vector.tensor_copy(sumsB[row:row + 1, :], psh[h][DK:DK + 1, :])
                for h in range(2):
                    key = (pp, j, h)
                    hu[key] = hupool.tile([DK, NQ], bf16, tag="hu", name=f"hu_{pp}_{j}_{h}")
                    if j == nb - 1 and pp == npair - 1:
                        # tail: DVE is the bottleneck there, ACT is idle
                        nc.scalar.copy(hu[key][:], psh[h][0:DK, :])
                    else:
                        nc.vector.tensor_copy(hu[key][:], psh[h][0:DK, :])

            released_x = False
            for j in range(nb):
                filler = []
                if j > 0:
                    # this block's own later v tiles (needed from m = dpb*j)
                    for tk in range(dpb * j, min(dpb * (j + 1), tt)):
                        filler.append(lambda tk=tk: emit_v(tk))
                if j == 0:
                    # kq(pp, n) must complete before block n starts
                    for pp in range(1, npair):
                        filler.append(lambda pp=pp: emit_kq(pp, 0, (0,)))
                        filler.append(lambda pp=pp: emit_kq(pp, 0, (1,)))
                if j < nb - 1:
                    for pp in range(npair):
                        filler.append(lambda pp=pp, n=j + 1: emit_kq(pp, n, (0,)))
                        filler.append(lambda pp=pp, n=j + 1: emit_kq(pp, n, (1,)))
                if j == nb - 1:
                    if nslotsA:
                        filler.append(lambda: emit_normA_recip(nslotsA))
                        # per q-block: its 4 normalization slots, then the
                        # output-projection token blocks they unlock
                        for b in range(nb - 1):
                            for i in range(4 * b, 4 * b + 4):
                                filler.append(lambda i=i: emit_normA_slot(i, nslotsA))
                            for tk in range(dpb * b, dpb * (b + 1)):
                                filler.append(lambda tk=tk: emit_oproj(tk))
                nm = dpb * (j + 1)
                nsteps = npair * nm
                fstate = [0, 0, nsteps]  # steps done, fillers emitted, total
                for pp in range(npair):
                    psh = [psH.tile([P, NQ], f32, tag="h", name=f"psh_{pp}_{j}_{h}") for h in range(2)]
                    pending = None
                    for m in range(nm):
                        step = emit_step(pp, j, m)
                        if pending is not None:
                            _emit_pv(pp, psh, nm, *pending)
                        pending = step
                        fstate[0] += 1
                        while fstate[1] < len(filler) and fstate[1] < fstate[0] * len(filler) // max(fstate[2], 1):
                            filler[fstate[1]]()
                            fstate[1] += 1
                    _emit_pv(pp, psh, nm, *pending)
                    finish_block(pp, j, psh)
                    if j == nb - 1 and pp < npair - 1:
                        # queue this pair's normalization as filler so it
                        # drips into the next pair's steps
                        filler.append(lambda pp=pp: emit_normB(batchB[2 * pp:2 * pp + 2], 2 * pp))
                while fstate[1] < len(filler):
                    filler[fstate[1]]()
                    fstate[1] += 1
                if j >= nb - 2 and not released_x:
                    released_x = True
                    xpool.release()

            # ---- tail: last pair's normalization + last block's output
            # projection, software-pipelined 4 psum buffers deep. The first
            # pair-0 half-accumulations issue before the normalization chain
            # so the PE stays busy (and HAM-warm) while it resolves.
            units = [(tk, ob) for tk in range(dpb * (nb - 1), tt) for ob in range(d // obw)]
            psys = {}

            def tail_p0(u):
                tk, ob = units[u]
                pl, tg, w = ((psF, "acc", obw), (psH, "h", NQ))[u % 2]
                psy = pl.tile([P, w], f32, tag=tg, name=f"yt_{u}")
                nc.tensor.matmul(
                    psy[:, 0:obw],
                    hT[0][nb - 1][:, (tk % dpb) * P:(tk % dpb + 1) * P],
                    wo_sb[:, ob * obw:(ob + 1) * obw],
                    start=True, stop=False,
                )
                psys[u] = psy

            def tail_p1(u):
                tk, ob = units[u]
                psy = psys.pop(u)
                nc.tensor.matmul(
                    psy[:, 0:obw],
                    hT[1][nb - 1][:, (tk % dpb) * P:(tk % dpb + 1) * P],
                    wo_sb[:, d + ob * obw: d + (ob + 1) * obw],
                    start=False, stop=True,
                )
                ysb = wpool.tile([P, obw], bf16, tag="ysb", bufs=4)
                nc.scalar.copy(ysb[:], psy[:, 0:obw])
                nc.sync.dma_start(y[tk * P:(tk + 1) * P, ob * obw:(ob + 1) * obw], ysb[:])

            for u in range(4):
                tail_p0(u)
            emit_normB(batchB[2 * (npair - 1):2 * (npair - 1) + 2], 2 * (npair - 1), ps_pool=psS)
            for u in range(4):
                tail_p1(u)
            for u in range(4, len(units)):
                tail_p0(u)
            for u in range(4, len(units)):
                tail_p1(u)

    nc.compile()
    return nc


def _get_nc():
    if "nc" not in _CACHE:
        _CACHE["nc"] = _build_nc()
    return _CACHE["nc"]


def _emat_np(nrows):
    import ml_dtypes
    e = np.zeros((nrows, nrows * DK), ml_dtypes.bfloat16)
    for i in range(nrows):
        e[i, i * DK:(i + 1) * DK] = 1.0
    return e


def _masks_np():
    import ml_dtypes
    kk = np.arange(P)[:, None]
    qq = np.arange(P)[None, :]
    return (kk <= qq).astype(ml_dtypes.bfloat16)


def _emat_rows(t=T, od=OD):
    nb = t // NQ
    npair = od // P
    return max(2 * npair * (nb - 1), 1)


def _ematB_np(t=T, od=OD):
    nslotsB = 2 * (od // P)
    nrowsB = 32 * (nslotsB - 1) + 1
    e = np.zeros((nrowsB, nslotsB * DK), np.float32)
    for i in range(nslotsB):
        e[32 * i, i * DK:(i + 1) * DK] = 1.0
    return e


def make_in_maps(x, Wq, Wk, Wv, Wo):
    import ml_dtypes

    bf = ml_dtypes.bfloat16
    x = np.asarray(x, np.float32)
    msk = _masks_np()
    emat = _emat_np(_emat_rows())
    in_maps = []
    for c in range(N_CORES):
        b, g = c // (N_CORES // 2), c % (N_CORES // 2)
        hs = slice(OD * g, OD * (g + 1))
        in_maps.append({
            "xT": np.ascontiguousarray(x[b].T).astype(bf),
            "wqT": np.ascontiguousarray(np.asarray(Wq, np.float32)[hs, :].T).astype(bf),
            "wkT": np.ascontiguousarray(np.asarray(Wk, np.float32)[hs, :].T).astype(bf),
            "wvT": np.ascontiguousarray(np.asarray(Wv, np.float32)[hs, :].T).astype(bf),
            "woT": np.ascontiguousarray(np.asarray(Wo, np.float32)[:, hs].T).astype(bf),
            "masks": msk,
            "emat": emat,
            "ematB": _ematB_np(),
        })
    return in_maps


def combine_outputs(results):
    ng = N_CORES // 2
    out = np.empty((2, T, D), np.float32)
    for b in range(2):
        acc = results[b * ng]["y"].astype(np.float32)
        for g in range(1, ng):
            acc = acc + results[b * ng + g]["y"].astype(np.float32)
        out[b] = acc
    return out


def kernel(x, Wq, Wk, Wv, Wo):
    from concourse.bass_utils import run_bass_kernel_spmd

    nc = _get_nc()
    in_maps = make_in_maps(x, Wq, Wk, Wv, Wo)
    res = run_bass_kernel_spmd(nc, in_maps, list(range(N_CORES)))
    return combine_outputs(res.results)
